# revision 1
# baseline (speedup 1.0000x reference)
"""Trainium2 Bass kernel: LayerNorm + QKV projection + RoPE (dense transformer).

Full inputs in, full outputs out. Internally shards the 8192 token rows
(b=2 x n=4096) across 8 NeuronCores (data parallel, 1024 tokens/core).

Per-core pipeline:
  1. DMA x tile [128, 2048]; LayerNorm stats via bn_stats/bn_aggr;
     xn = (x - mu) * rsqrt(var + eps)          (VectorE)
  2. PE-transpose xn 128x128 blocks -> PSUM; ScalarE Identity-copy to SBUF
     casting to the matmul dtype and applying gamma/beta (per-partition
     scale/bias in transposed layout)
  3. QKV matmuls (fp16 by default: ~tf32 accuracy at 2-byte cost):
     out[t, e] accumulated over 16 k-tiles in PSUM; weights streamed as
     half-matrix chunks [128, 16, 1024], double buffered
  4. RoPE on q (VectorE) / k (GPSIMD) with host-precomputed cos/sin tables
  5. DMA out contiguous row blocks; host re-assembles [b, h, n, hd]
"""

import os
from contextlib import ExitStack

import numpy as np

import concourse.bass as bass
import concourse.tile as tile
from concourse import bacc, mybir
from concourse.bass_utils import run_bass_kernel_spmd
from concourse.masks import make_identity

# Problem shapes (hardcoded per contract)
B, N, DM = 2, 4096, 2048
NCORES = 8
TOK = B * N            # 8192 total token rows
TPC = TOK // NCORES    # 1024 tokens per core
P = 128
MT = TPC // P          # 8 m-tiles per core
KT = DM // P           # 16 k-tiles (contraction)
HEADS, HD = 16, 128
ECW = 1024             # weight-chunk width (half the e range)
NCH = DM // ECW        # 2 chunks
NB = ECW // 512        # matmul n-slices per chunk (PSUM bank = 512 fp32)
HPC = ECW // HD        # heads per chunk = 8
LN_EPS = 1e-5
ROPE_BASE = 10000.0

F32 = mybir.dt.float32
# Matmul input dtype: float16 (default; ~tf32 accuracy) or bfloat16 (faster,
# lower accuracy). Must be a 2-byte dtype.
MM_DT = getattr(mybir.dt, os.environ.get("QKV_MM_DT", "float16"))

_CACHE = {}


def _build_nc(body_reps=None):
    if body_reps is None:
        body_reps = int(os.environ.get("QKV_BODY_REPS", "1"))
    nc = bacc.Bacc("TRN2", target_bir_lowering=False, debug=False,
                   enable_asserts=False, num_devices=NCORES)

    x = nc.dram_tensor("x", [TPC, DM], F32, kind="ExternalInput").ap()
    wts = [
        nc.dram_tensor(f"w{n}", [KT, P, DM], MM_DT,
                       kind="ExternalInput").ap()
        for n in "qkv"
    ]
    gammaT = nc.dram_tensor("gammaT", [P, KT], F32, kind="ExternalInput").ap()
    betaT = nc.dram_tensor("betaT", [P, KT], F32, kind="ExternalInput").ap()
    cosT = nc.dram_tensor("cosT", [P, MT, HD // 2], F32, kind="ExternalInput").ap()
    sinT = nc.dram_tensor("sinT", [P, MT, HD // 2], F32, kind="ExternalInput").ap()
    outs = [
        nc.dram_tensor(f"{n}_out", [TPC, DM], F32, kind="ExternalOutput").ap()
        for n in "qkv"
    ]

    with tile.TileContext(nc) as tc:
        for _rep in range(body_reps):
            with ExitStack() as ctx:
                _kernel_body(ctx, tc, x, wts, gammaT, betaT, cosT, sinT, outs)
    nc.compile()
    return nc


def _kernel_body(ctx, tc, x, wts, gammaT, betaT, cosT, sinT, outs):
    nc = tc.nc

    singles = ctx.enter_context(tc.tile_pool(name="singles", bufs=1))
    xpool = ctx.enter_context(tc.tile_pool(name="xpool", bufs=3))
    stats_pool = ctx.enter_context(tc.tile_pool(name="stats", bufs=4))
    xnt_pool = ctx.enter_context(tc.tile_pool(name="xnt", bufs=1))
    wt_pool = ctx.enter_context(tc.tile_pool(name="wt", bufs=2))
    stage_pool = ctx.enter_context(tc.tile_pool(name="stage", bufs=4))
    rope_pool = ctx.enter_context(tc.tile_pool(name="rope", bufs=3))
    # One shared PSUM pool (8 banks): phase A transposes + phase B accums
    psum = ctx.enter_context(tc.tile_pool(name="psum", bufs=8, space="PSUM"))

    # One-time constants
    identity = singles.tile([P, P], F32)
    make_identity(nc, identity)
    eps_t = singles.tile([P, 1], F32)
    nc.vector.memset(eps_t, LN_EPS)
    gamma_sb = singles.tile([P, KT], F32)
    nc.sync.dma_start(out=gamma_sb, in_=gammaT)
    beta_sb = singles.tile([P, KT], F32)
    nc.sync.dma_start(out=beta_sb, in_=betaT)
    cos_sb = singles.tile([P, MT, HD // 2], F32)
    nc.sync.dma_start(out=cos_sb, in_=cosT)
    sin_sb = singles.tile([P, MT, HD // 2], F32)
    nc.sync.dma_start(out=sin_sb, in_=sinT)

    # Persistent transposed normalized activations: [p=d_inner, k, t]
    xnt = xnt_pool.tile([P, KT, TPC], MM_DT)

    # ---- Phase A: LayerNorm + transpose, per m-tile ----
    for m in range(MT):
        x_t = xpool.tile([P, DM], F32)
        nc.sync.dma_start(out=x_t, in_=x[m * P:(m + 1) * P, :])

        xg = x_t.rearrange("p (g s) -> p g s", s=512)
        st = stats_pool.tile([P, 4, nc.vector.BN_STATS_DIM], F32)
        for g in range(4):
            nc.vector.bn_stats(out=st[:, g, :], in_=xg[:, g, :])
        mv = stats_pool.tile([P, nc.vector.BN_AGGR_DIM], F32)
        nc.vector.bn_aggr(out=mv, in_=st)

        # rsig = 1/sqrt(var + eps)
        rsig = stats_pool.tile([P, 1], F32)
        nc.scalar.activation(out=rsig, in_=mv[:, 1:2],
                             func=mybir.ActivationFunctionType.Sqrt,
                             bias=eps_t, scale=1.0)
        nc.vector.reciprocal(out=rsig, in_=rsig)

        # xn = (x - mu) * rsig (in place)
        nc.vector.tensor_scalar(out=x_t, in0=x_t,
                                scalar1=mv[:, 0:1], scalar2=rsig,
                                op0=mybir.AluOpType.subtract,
                                op1=mybir.AluOpType.mult)

        # Transpose each 128x128 block; apply gamma/beta + cast to MM_DT
        # during the PSUM->SBUF copy
        for k in range(KT):
            pt = psum.tile([P, 512], F32, space="PSUM", name="ps")
            nc.tensor.transpose(pt[:, 0:P], x_t[:, k * P:(k + 1) * P],
                                identity)
            nc.scalar.activation(out=xnt[:, k, m * P:(m + 1) * P],
                                 in_=pt[:, 0:P],
                                 func=mybir.ActivationFunctionType.Identity,
                                 bias=beta_sb[:, k:k + 1],
                                 scale=gamma_sb[:, k:k + 1])

    # ---- Phase B: QKV matmuls + RoPE + store ----
    for wi, (w_dram, o_dram) in enumerate(zip(wts, outs)):
        for c in range(NCH):
            w_sb = wt_pool.tile([P, KT, ECW], MM_DT)
            for k in range(KT):
                nc.sync.dma_start(out=w_sb[:, k, :],
                                  in_=w_dram[k, :, c * ECW:(c + 1) * ECW])

            for m in range(MT):
                accs = [psum.tile([P, 512], F32, space="PSUM", name="ps")
                        for _ in range(NB)]
                for k in range(KT):
                    lhsT = xnt[:, k, m * P:(m + 1) * P]
                    for n in range(NB):
                        nc.tensor.matmul(
                            accs[n], lhsT=lhsT,
                            rhs=w_sb[:, k, n * 512:(n + 1) * 512],
                            start=(k == 0), stop=(k == KT - 1),
                        )

                stg = stage_pool.tile([P, ECW], F32)
                for n in range(NB):
                    nc.scalar.activation(
                        out=stg[:, n * 512:(n + 1) * 512], in_=accs[n],
                        func=mybir.ActivationFunctionType.Copy)

                if wi < 2:  # rope on q and k
                    eng = nc.vector if wi == 0 else nc.gpsimd
                    ov = stg.rearrange("p (h d) -> p h d", d=HD)
                    q1 = ov[:, :, 0:HD // 2]
                    q2 = ov[:, :, HD // 2:HD]
                    cos_m = cos_sb[:, m, :]
                    sin_m = sin_sb[:, m, :]
                    cos_b = bass.AP(tensor=cos_m.tensor, offset=cos_m.offset,
                                    ap=[cos_m.ap[0], [0, HPC], cos_m.ap[1]])
                    sin_b = bass.AP(tensor=sin_m.tensor, offset=sin_m.offset,
                                    ap=[sin_m.ap[0], [0, HPC], sin_m.ap[1]])
                    ta = rope_pool.tile([P, HPC, HD // 2], F32,
                                        name=f"ropeA{wi}")
                    tb = rope_pool.tile([P, HPC, HD // 2], F32,
                                        name=f"ropeB{wi}")
                    eng.tensor_mul(ta, q1, sin_b)      # A = q1*sin
                    eng.tensor_mul(tb, q2, sin_b)      # B = q2*sin
                    eng.tensor_mul(q1, q1, cos_b)      # q1 = q1*cos
                    eng.tensor_sub(q1, q1, tb)         # q1 -= B
                    eng.tensor_mul(q2, q2, cos_b)      # q2 = q2*cos
                    eng.tensor_add(q2, q2, ta)         # q2 += A

                nc.sync.dma_start(
                    out=o_dram[m * P:(m + 1) * P, c * ECW:(c + 1) * ECW],
                    in_=stg)


def _host_prep(x, ln_gamma, ln_beta, wq, wk, wv):
    """Shard/layout inputs. Returns per-core input maps."""
    xf = np.ascontiguousarray(x.reshape(TOK, DM), dtype=np.float32)
    wdt = mybir.dt.np(MM_DT)

    def tile_w(w):
        wt = np.asarray(w, np.float32).T  # [d, e]
        return np.ascontiguousarray(wt.reshape(KT, P, DM)).astype(wdt)

    wq_t, wk_t, wv_t = tile_w(wq), tile_w(wk), tile_w(wv)
    gammaT = np.ascontiguousarray(
        np.asarray(ln_gamma, np.float32).reshape(KT, P).T)
    betaT = np.ascontiguousarray(
        np.asarray(ln_beta, np.float32).reshape(KT, P).T)

    # Build RoPE tables with jax.numpy, matching the reference's fp32 trig
    # bit-for-bit (numpy's fp32 cos differs by ~3e-4 at large arguments).
    import jax.numpy as jnp
    inv_freq = 1.0 / (ROPE_BASE ** (jnp.arange(0, HD, 2, dtype=jnp.float32) / HD))
    t = jnp.arange(N, dtype=jnp.float32)
    freqs = jnp.einsum("i,j->ij", t, inv_freq)  # [N, 64]
    cos_full = np.asarray(jnp.cos(freqs), dtype=np.float32)
    sin_full = np.asarray(jnp.sin(freqs), dtype=np.float32)

    in_maps = []
    for c in range(NCORES):
        pos0 = (c * TPC) % N
        cos_c = np.ascontiguousarray(
            cos_full[pos0:pos0 + TPC].reshape(MT, P, HD // 2).transpose(1, 0, 2))
        sin_c = np.ascontiguousarray(
            sin_full[pos0:pos0 + TPC].reshape(MT, P, HD // 2).transpose(1, 0, 2))
        in_maps.append({
            "x": np.ascontiguousarray(xf[c * TPC:(c + 1) * TPC]),
            "wq": wq_t, "wk": wk_t, "wv": wv_t,
            "gammaT": gammaT, "betaT": betaT,
            "cosT": cos_c, "sinT": sin_c,
        })
    return in_maps


def _assemble(res_list, name):
    full = np.concatenate([res_list[c][name] for c in range(NCORES)], axis=0)
    return np.ascontiguousarray(
        full.reshape(B, N, HEADS, HD).transpose(0, 2, 1, 3))


def kernel(x, ln_gamma, ln_beta, wq, wk, wv, num_heads, _trace=False):
    assert int(num_heads) == HEADS
    in_maps = _host_prep(x, ln_gamma, ln_beta, wq, wk, wv)
    if "nc" not in _CACHE:
        _CACHE["nc"] = _build_nc()
    nc = _CACHE["nc"]
    r = run_bass_kernel_spmd(nc, in_maps, core_ids=list(range(NCORES)),
                             trace=_trace)
    if _trace:
        _CACHE["last_results"] = r
    q = _assemble(r.results, "q_out")
    k = _assemble(r.results, "k_out")
    v = _assemble(r.results, "v_out")
    return q, k, v



# revision 16
# speedup vs baseline: 1.1778x; 1.1778x over previous
"""Trainium2 Bass kernel: LayerNorm + QKV projection + RoPE (dense transformer).

Full inputs in, full outputs out. Internally shards the 8192 token rows
(b=2 x n=4096) across 8 NeuronCores (data parallel, 1024 tokens/core).

v3 pipeline (PE does matmuls + cheap fp16 transposes; LN affine folded out):
  1. x tile [128, 2048] DMA'd in 4 column slices (SP queue); bn_stats per
     slice as it lands; bn_aggr -> mu/var; rsig = Rsqrt(var+eps) (ScalarE);
     nbias = -mu*rsig (VectorE)
  2. ScalarE activation: xn16 = MM_DT(rsig*x + nbias).  ln gamma is folded
     into the weights on the host (w' = gamma * w^T); ln beta contributes a
     constant row the host adds post-hoc (RoPE is linear), so the device
     never sees gamma/beta.
  3. PE-transpose the 16 128x128 fp16 blocks (1 cy/row) into two fp16 PSUM
     banks (8 blocks each); one wide copy per bank into xnt (VectorE for
     bank 0, ScalarE for bank 1)
  4. QKV matmuls: out[t, e] accumulated over 16 k-tiles in PSUM; weights
     streamed as half-matrix chunks [128, 16, 1024]; the first chunk's DMAs
     are emitted right after m-tile 0 so the PE has matmul work at t~6us
  5. PSUM -> SBUF copy (ScalarE); RoPE on q and k (VectorE); output stores
     issued from the ScalarE HWDGE queue so they never delay input loads.
"""

import os
from contextlib import ExitStack

import numpy as np

import concourse.bass as bass
import concourse.tile as tile
from concourse import bacc, mybir
from concourse.bass_utils import run_bass_kernel_spmd
from concourse.masks import make_identity

# Problem shapes (hardcoded per contract)
B, N, DM = 2, 4096, 2048
NCORES = 8
TOK = B * N            # 8192 total token rows
TPC = TOK // NCORES    # 1024 tokens per core
P = 128
MT = TPC // P          # 8 m-tiles per core
KT = DM // P           # 16 k-tiles (contraction)
HEADS, HD = 16, 128
ECW = 1024             # weight-chunk width (half the e range)
NCH = DM // ECW        # 2 chunks
NB = ECW // 512        # matmul n-slices per chunk (PSUM bank = 512 fp32)
HPC = ECW // HD        # heads per chunk = 8
XG = 4                 # x DMA column split (per-slice bn_stats)
KHALF = KT // 2        # transposed k-blocks per fp16 PSUM bank
LN_EPS = 1e-5
ROPE_BASE = 10000.0

F32 = mybir.dt.float32
# Matmul input dtype: float16 (default; ~tf32 accuracy) or bfloat16.
MM_DT = getattr(mybir.dt, os.environ.get("QKV_MM_DT", "float16"))

_CACHE = {}


def _build_nc(body_reps=None):
    if body_reps is None:
        body_reps = int(os.environ.get("QKV_BODY_REPS", "1"))
    nc = bacc.Bacc("TRN2", target_bir_lowering=False, debug=False,
                   enable_asserts=False, num_devices=NCORES)

    x = nc.dram_tensor("x", [TPC, DM], F32, kind="ExternalInput").ap()
    wts = [
        nc.dram_tensor(f"w{n}", [KT, P, DM], MM_DT,
                       kind="ExternalInput").ap()
        for n in "qkv"
    ]
    cosT = nc.dram_tensor("cosT", [P, MT, HD // 2], F32, kind="ExternalInput").ap()
    sinT = nc.dram_tensor("sinT", [P, MT, HD // 2], F32, kind="ExternalInput").ap()
    outs = [
        nc.dram_tensor(f"{n}_out", [TPC, DM], F32, kind="ExternalOutput").ap()
        for n in "qkv"
    ]

    with tile.TileContext(nc) as tc:
        for _rep in range(body_reps):
            with ExitStack() as ctx:
                _kernel_body(ctx, tc, x, wts, cosT, sinT, outs)
    nc.compile()
    return nc


def _kernel_body(ctx, tc, x, wts, cosT, sinT, outs):
    nc = tc.nc

    singles = ctx.enter_context(tc.tile_pool(name="singles", bufs=1))
    xpool = ctx.enter_context(tc.tile_pool(name="xpool", bufs=3))
    stats_pool = ctx.enter_context(tc.tile_pool(name="stats", bufs=4))
    xn16_pool = ctx.enter_context(tc.tile_pool(name="xn16", bufs=3))
    xnt_pool = ctx.enter_context(tc.tile_pool(name="xnt", bufs=1))
    wt_pool = ctx.enter_context(tc.tile_pool(name="wt", bufs=2))
    stage_pool = ctx.enter_context(tc.tile_pool(name="stage", bufs=4))
    rope_pool = ctx.enter_context(tc.tile_pool(name="rope", bufs=3))
    # PSUM: 2 banks cycle through phase-A fp16 transposes, 6 hold matmul
    # accumulation groups (3 blocks in flight)
    psumT = ctx.enter_context(tc.tile_pool(name="psumT", bufs=2, space="PSUM"))
    psumB = ctx.enter_context(tc.tile_pool(name="psumB", bufs=6, space="PSUM"))

    # One-time constants
    eps_t = singles.tile([P, 1], F32)
    nc.vector.memset(eps_t, LN_EPS)
    # Dummy activation at t=0 so the 1.3us act-table load overlaps the
    # first x DMA instead of sitting on m-tile 0's critical path.
    warm = singles.tile([P, 1], F32)
    nc.scalar.activation(out=warm, in_=eps_t,
                         func=mybir.ActivationFunctionType.Sqrt,
                         bias=eps_t, scale=1.0)
    identity = singles.tile([P, P], MM_DT)
    make_identity(nc, identity)
    cos_sb = singles.tile([P, MT, HD // 2], F32)
    sin_sb = singles.tile([P, MT, HD // 2], F32)

    # Persistent transposed normalized activations: [p=d_inner, k, t]
    xnt = xnt_pool.tile([P, KT, TPC], MM_DT)

    w_tiles = {}

    def load_chunk(wi, c, ks=None):
        if (wi, c) not in w_tiles:
            w_tiles[(wi, c)] = wt_pool.tile([P, KT, ECW], MM_DT,
                                            name="w_sb")
        w_sb = w_tiles[(wi, c)]
        for k in (range(KT) if ks is None else ks):
            nc.sync.dma_start(out=w_sb[:, k, :],
                              in_=wts[wi][k, :, c * ECW:(c + 1) * ECW])

    # ---- Phase A: LayerNorm + cast + PE transpose, per m-tile ----
    GW = DM // XG
    for m in range(MT):
        x_t = xpool.tile([P, DM], F32)
        st = stats_pool.tile([P, XG, nc.vector.BN_STATS_DIM], F32)
        for g in range(XG):
            sl = slice(g * GW, (g + 1) * GW)
            nc.sync.dma_start(out=x_t[:, sl], in_=x[m * P:(m + 1) * P, sl])
            nc.vector.bn_stats(out=st[:, g, :], in_=x_t[:, sl])
        mv = stats_pool.tile([P, nc.vector.BN_AGGR_DIM], F32)
        nc.vector.bn_aggr(out=mv, in_=st)

        # rsig = 1/sqrt(var + eps); nbias = -mu*rsig
        rsig = stats_pool.tile([P, 2], F32)
        nc.scalar.activation(out=rsig[:, 0:1], in_=mv[:, 1:2],
                             func=mybir.ActivationFunctionType.Sqrt,
                             bias=eps_t, scale=1.0)
        nc.vector.reciprocal(out=rsig[:, 0:1], in_=rsig[:, 0:1])
        nc.vector.tensor_scalar(out=rsig[:, 1:2], in0=rsig[:, 0:1],
                                scalar1=mv[:, 0:1], scalar2=-1.0,
                                op0=mybir.AluOpType.mult,
                                op1=mybir.AluOpType.mult)

        # xn16 = MM_DT(rsig*x + nbias), in halves so bank-0 transposes start
        # a pass earlier; then 8 fp16 transposes per PSUM bank + 1 wide copy
        xn16 = xn16_pool.tile([P, DM], MM_DT)
        HW_ = KHALF * P
        for h in range(2):
            nc.scalar.activation(out=xn16[:, h * HW_:(h + 1) * HW_],
                                 in_=x_t[:, h * HW_:(h + 1) * HW_],
                                 func=mybir.ActivationFunctionType.Identity,
                                 bias=rsig[:, 1:2], scale=rsig[:, 0:1])
            pt = psumT.tile([P, HW_], MM_DT, space="PSUM", name="pt")
            for j in range(KHALF):
                k = h * KHALF + j
                nc.tensor.transpose(pt[:, j * P:(j + 1) * P],
                                    xn16[:, k * P:(k + 1) * P], identity)
            src = pt.rearrange("p (k t) -> p k t", t=P)
            dst = xnt[:, h * KHALF:(h + 1) * KHALF, m * P:(m + 1) * P]
            # Both on ScalarE: GPSIMD has no PSUM port, and VectorE must
            # stay free for LN stats + the q-RoPE that overlaps phase A
            nc.scalar.copy(dst, src)

        # First weight chunk's k-slices interleave with the early x tiles
        # on the DMA stream: the PE gets matmul work at t ~ 7us without
        # starving phase A of x bandwidth. cos/sin ride in the middle
        # (first needed by RoPE at t ~ 20us).
        if m == 0:
            load_chunk(0, 0, range(0, 4))
        elif m == 1:
            load_chunk(0, 0, range(4, 10))
        elif m == 2:
            load_chunk(0, 0, range(10, KT))
            nc.sync.dma_start(out=cos_sb, in_=cosT)
            nc.sync.dma_start(out=sin_sb, in_=sinT)

    # ---- Phase B: QKV matmuls + RoPE + store ----
    for wi, (w_dram, o_dram) in enumerate(zip(wts, outs)):
        for c in range(NCH):
            if (wi, c) not in w_tiles:
                load_chunk(wi, c)
            w_sb = w_tiles.pop((wi, c))

            for m in range(MT):
                accs = [psumB.tile([P, 512], F32, space="PSUM", name="ps")
                        for _ in range(NB)]
                final = (wi == 2 and c == NCH - 1 and m == MT - 1)
                if final:
                    # n-major: bank 0 finishes its accumulation ~3.4us
                    # before bank 1, so half the output drains early and
                    # the program tail is one bank's copy+store
                    for n in range(NB):
                        for k in range(KT):
                            nc.tensor.matmul(
                                accs[n], lhsT=xnt[:, k, m * P:(m + 1) * P],
                                rhs=w_sb[:, k, n * 512:(n + 1) * 512],
                                start=(k == 0), stop=(k == KT - 1),
                            )
                else:
                    for k in range(KT):
                        lhsT = xnt[:, k, m * P:(m + 1) * P]
                        for n in range(NB):
                            nc.tensor.matmul(
                                accs[n], lhsT=lhsT,
                                rhs=w_sb[:, k, n * 512:(n + 1) * 512],
                                start=(k == 0), stop=(k == KT - 1),
                            )

                stg = stage_pool.tile([P, ECW], F32)
                if wi == 2:
                    # v: no RoPE; copy+store per 512-half to shorten the
                    # tail and overlap copy with DMA. The very last block
                    # splits across engines/queues so the two halves drain
                    # in parallel.
                    last = (c == NCH - 1 and m == MT - 1)
                    if last:
                        # quarter-stores across two engines and two HWDGE
                        # queues: the program tail is one 256-col drain
                        for j in range(4):
                            qs = stg[:, j * 256:(j + 1) * 256]
                            o_sl = o_dram[m * P:(m + 1) * P,
                                          c * ECW + j * 256:
                                          c * ECW + (j + 1) * 256]
                            src = accs[j // 2][:, (j % 2) * 256:
                                               (j % 2 + 1) * 256]
                            if j % 2 == 0:
                                nc.scalar.activation(
                                    out=qs, in_=src,
                                    func=mybir.ActivationFunctionType.Copy)
                                nc.scalar.dma_start(out=o_sl, in_=qs)
                            else:
                                nc.vector.tensor_copy(qs, src)
                                nc.sync.dma_start(out=o_sl, in_=qs)
                    else:
                        for n in range(NB):
                            half = stg[:, n * 512:(n + 1) * 512]
                            o_sl = o_dram[m * P:(m + 1) * P,
                                          c * ECW + n * 512:
                                          c * ECW + (n + 1) * 512]
                            nc.scalar.activation(
                                out=half, in_=accs[n],
                                func=mybir.ActivationFunctionType.Copy)
                            nc.scalar.dma_start(out=o_sl, in_=half)
                else:
                    for n in range(NB):
                        nc.scalar.activation(
                            out=stg[:, n * 512:(n + 1) * 512], in_=accs[n],
                            func=mybir.ActivationFunctionType.Copy)
                    # RoPE on VectorE for both q and k
                    eng = nc.vector
                    ov = stg.rearrange("p (h d) -> p h d", d=HD)
                    q1 = ov[:, :, 0:HD // 2]
                    q2 = ov[:, :, HD // 2:HD]
                    cos_m = cos_sb[:, m, :]
                    sin_m = sin_sb[:, m, :]
                    cos_b = bass.AP(tensor=cos_m.tensor, offset=cos_m.offset,
                                    ap=[cos_m.ap[0], [0, HPC], cos_m.ap[1]])
                    sin_b = bass.AP(tensor=sin_m.tensor, offset=sin_m.offset,
                                    ap=[sin_m.ap[0], [0, HPC], sin_m.ap[1]])
                    ta = rope_pool.tile([P, HPC, HD // 2], F32,
                                        name=f"ropeA{wi}")
                    tb = rope_pool.tile([P, HPC, HD // 2], F32,
                                        name=f"ropeB{wi}")
                    eng.tensor_mul(ta, q1, sin_b)      # A = q1*sin
                    eng.tensor_mul(tb, q2, sin_b)      # B = q2*sin
                    eng.tensor_mul(q1, q1, cos_b)      # q1 = q1*cos
                    eng.tensor_sub(q1, q1, tb)         # q1 -= B
                    eng.tensor_mul(q2, q2, cos_b)      # q2 = q2*cos
                    eng.tensor_add(q2, q2, ta)         # q2 += A

                    # Store from the ScalarE HWDGE queue; loads own SP
                    nc.scalar.dma_start(
                        out=o_dram[m * P:(m + 1) * P,
                                   c * ECW:(c + 1) * ECW],
                        in_=stg)

            # Prefetch the next chunk behind this one's last m-tile
            nxt = (wi, c + 1) if c + 1 < NCH else (wi + 1, 0)
            if nxt[0] < 3 and nxt not in w_tiles:
                load_chunk(*nxt)


def _rope_tables():
    # Built with jax.numpy, matching the reference's fp32 trig bit-for-bit
    # (numpy's fp32 cos differs by ~3e-4 at large arguments).
    import jax.numpy as jnp
    inv_freq = 1.0 / (ROPE_BASE ** (jnp.arange(0, HD, 2, dtype=jnp.float32) / HD))
    t = jnp.arange(N, dtype=jnp.float32)
    freqs = jnp.einsum("i,j->ij", t, inv_freq)  # [N, 64]
    cos_full = np.asarray(jnp.cos(freqs), dtype=np.float32)
    sin_full = np.asarray(jnp.sin(freqs), dtype=np.float32)
    return cos_full, sin_full


def _host_prep(x, ln_gamma, ln_beta, wq, wk, wv):
    """Shard/layout inputs. Returns per-core input maps."""
    xf = np.ascontiguousarray(x.reshape(TOK, DM), dtype=np.float32)
    wdt = mybir.dt.np(MM_DT)
    gamma = np.asarray(ln_gamma, np.float32)

    def tile_w(w):
        # w stored [out, in]; fold ln gamma into w^T rows: w'[d, e]
        wt = np.asarray(w, np.float32).T * gamma[:, None]
        return np.ascontiguousarray(wt.reshape(KT, P, DM)).astype(wdt)

    wq_t, wk_t, wv_t = tile_w(wq), tile_w(wk), tile_w(wv)
    cos_full, sin_full = _rope_tables()

    in_maps = []
    for c in range(NCORES):
        pos0 = (c * TPC) % N
        cos_c = np.ascontiguousarray(
            cos_full[pos0:pos0 + TPC].reshape(MT, P, HD // 2).transpose(1, 0, 2))
        sin_c = np.ascontiguousarray(
            sin_full[pos0:pos0 + TPC].reshape(MT, P, HD // 2).transpose(1, 0, 2))
        in_maps.append({
            "x": np.ascontiguousarray(xf[c * TPC:(c + 1) * TPC]),
            "wq": wq_t, "wk": wk_t, "wv": wv_t,
            "cosT": cos_c, "sinT": sin_c,
        })
    return in_maps


def _assemble(res_list, name):
    full = np.concatenate([res_list[c][name] for c in range(NCORES)], axis=0)
    return np.ascontiguousarray(
        full.reshape(B, N, HEADS, HD).transpose(0, 2, 1, 3))


def _beta_correction(ln_gamma, ln_beta, w):
    """Constant row the device omitted: beta @ (gamma * w^T), as [e]."""
    gamma = np.asarray(ln_gamma, np.float64)
    beta = np.asarray(ln_beta, np.float64)
    wt = np.asarray(w, np.float64).T * gamma[:, None]
    return (beta @ wt).astype(np.float32)


def kernel(x, ln_gamma, ln_beta, wq, wk, wv, num_heads, _trace=False):
    assert int(num_heads) == HEADS
    in_maps = _host_prep(x, ln_gamma, ln_beta, wq, wk, wv)
    if "nc" not in _CACHE:
        _CACHE["nc"] = _build_nc()
    nc = _CACHE["nc"]
    r = run_bass_kernel_spmd(nc, in_maps, core_ids=list(range(NCORES)),
                             trace=_trace)
    if _trace:
        _CACHE["last_results"] = r
    q = _assemble(r.results, "q_out")
    k = _assemble(r.results, "k_out")
    v = _assemble(r.results, "v_out")

    if np.any(np.asarray(ln_beta) != 0):
        # RoPE is linear: R(q + b) = R(q) + R(b). Add the rotated constant
        # row on the host (never taken for this problem's zero beta).
        cos_full, sin_full = _rope_tables()  # [N, 64]
        cosn = np.concatenate([cos_full, cos_full], axis=1)  # [N, 128]
        sinn = np.concatenate([sin_full, sin_full], axis=1)
        for arr, w in ((q, wq), (k, wk)):
            b_row = _beta_correction(ln_gamma, ln_beta, w).reshape(HEADS, HD)
            rot = np.concatenate([-b_row[:, HD // 2:], b_row[:, :HD // 2]], 1)
            corr = (b_row[None, :, :] * cosn[:, None, :]
                    + rot[None, :, :] * sinn[:, None, :])  # [N, H, HD]
            arr += corr.transpose(1, 0, 2)[None]
        v += _beta_correction(ln_gamma, ln_beta, wv).reshape(
            1, HEADS, 1, HD)
    return q, k, v
